# revision 1
# baseline (speedup 1.0000x reference)
"""Trainium2 Bass kernel for nn_HHGNN_17179869473.

Computation (per batch b):
    y     = embeds @ W[t_n] + b[t_n]         (per-type linear, routed by type)
    x     = where(mask, y, embeds); x = LayerNorm(x) * gamma + beta
    lat   = adj^T @ x                        [M, H]
    ret   = adj @ lat                        [N, H]

Strategy: pure data parallel, batch b -> core b (B == 8 == n_cores).

Host-side prep per batch:
  * stable-sort nodes by type. Each 128-node block then holds <= S (~2)
    distinct types.
  * P1 select-free projection: the per-slot matmuls accumulate directly in
    PSUM using slot-GATED stationaries (host zeroes the columns of embT that
    don't belong to the slot's type), plus a 13-deep one-hot matmul that adds
    the routed bias.  No element-wise merge work on DVE at all.
  * adj is streamed as float8e3 (e3m4, 1 byte) in both layouts, column-mean
    centered on the host; the exact rank-1 mean term is restored on device by
    a cheap outer-product matmul folded into each PSUM accumulation chain.
    (Mixed-dtype matmul fp16 stationary x fp8e3 moving runs at full PE rate
    and was verified bit-accurate on hardware.)
  * fp32 accumulation in PSUM everywhere; x / lat / output in fp16.

Device phases (per core):
  P1: 3 matmuls per block (bias + S gated slots) -> x_pre fp16; LayerNorm
      stats batched over groups of 8 blocks on DVE; 2 stt ops per block.
  P2: latT[h,m] += x_blk^T @ adjP_blk (e3m4), + colsum_x outer mu (exact
      mean restore).  colsum_x computed by ones-stationary matmuls.
  P3: lat blocks PE-transposed, then retT[h,n] = nu-outer-latcolsum (chain
      start) + sum_mb lat_mb^T @ adjT_mb.  retT -> fp16 -> DRAM.
Host post: out[b] = unpermute(retT.T).
"""

import os
import sys

for _p in ("/opt/trn_rl_repo", "/root/.axon_site/_ro/trn_rl_repo"):
    if os.path.isdir(_p) and _p not in sys.path:
        sys.path.insert(0, _p)

import numpy as np
import ml_dtypes

import concourse.bacc as bacc
import concourse.tile as tile
from concourse import mybir
from concourse.bass_interp import get_hw_module
from concourse.bass_utils import run_bass_kernel_spmd

B, N, M, H, T = 8, 4096, 2048, 128, 13
NBLK = N // 128          # 32 node blocks
MBLK = M // 128          # 16 m blocks
MCH = M // 512           # 4 chunks over m
NCH = N // 512           # 8 chunks over n
GRP = 8                  # blocks per LayerNorm-stats group
NG = NBLK // GRP
LN_EPS = 1e-5
F32 = mybir.dt.float32
F16 = mybir.dt.float16
F8E3 = mybir.dt.float8e3
ADD = mybir.AluOpType.add
SUB = mybir.AluOpType.subtract
MUL = mybir.AluOpType.mult
AF = mybir.ActivationFunctionType
E3 = ml_dtypes.float8_e3m4

_PROGRAM_CACHE = {}


def _build_program(S, unmask):
    """SPMD-uniform Bass program. S = type slots per block; unmask = True
    adds the (1-mask)*embeds passthrough term."""
    nc = bacc.Bacc(
        "TRN2",
        target_bir_lowering=False,
        debug=False,
        enable_asserts=False,
        num_devices=B,
    )

    embTs_d = nc.dram_tensor("embTs", [128, S * N], F16, kind="ExternalInput").ap()
    Wblk_d = nc.dram_tensor("Wblk", [128, NBLK * S * 128], F16, kind="ExternalInput").ap()
    oneh_d = nc.dram_tensor("oneh", [T, N], F16, kind="ExternalInput").ap()
    btab_d = nc.dram_tensor("btab", [T, H], F16, kind="ExternalInput").ap()
    gb_d = nc.dram_tensor("gb", [128, H], F16, kind="ExternalInput").ap()
    bb_d = nc.dram_tensor("bb", [128, H], F16, kind="ExternalInput").ap()
    adjP_d = nc.dram_tensor("adjP", [128, NBLK * M], F8E3, kind="ExternalInput").ap()
    adjT_d = nc.dram_tensor("adjT", [128, MBLK * N], F8E3, kind="ExternalInput").ap()
    muB_d = nc.dram_tensor("muB", [1, M], F16, kind="ExternalInput").ap()
    nuB_d = nc.dram_tensor("nuB", [1, N], F16, kind="ExternalInput").ap()
    id_d = nc.dram_tensor("ident", [128, 128], F16, kind="ExternalInput").ap()
    if unmask:
        um_d = nc.dram_tensor("umT", [128, N], F16, kind="ExternalInput").ap()
    ret_d = nc.dram_tensor("retT", [128, N], F16, kind="ExternalOutput").ap()

    with tile.TileContext(nc, trace_sim=False) as tc:
        with (
            tc.tile_pool(name="const", bufs=1) as constp,
            tc.tile_pool(name="xpool", bufs=1) as xpool,
            tc.tile_pool(name="sqp", bufs=2) as sqp,
            tc.tile_pool(name="atp", bufs=4) as atp,
            tc.tile_pool(name="adjp", bufs=4) as adjp,
            tc.tile_pool(name="adjtp", bufs=6) as adjtp,
            tc.tile_pool(name="outp", bufs=8) as outp,
        ):
            # ---- constants: group-0 critical path on the sync (SP) HWDGE
            # queue, the rest on the SWDGE (gpsimd) queue ----
            CW = GRP * 128
            oneh_sb = constp.tile([T, N], F16)
            nc.sync.dma_start(oneh_sb[:], oneh_d)
            btab_sb = constp.tile([T, H], F16)
            nc.sync.dma_start(btab_sb[:], btab_d)
            embTs_sb = constp.tile([128, S * N], F16)
            Wblk_sb = constp.tile([128, NBLK * S * 128], F16)
            if unmask:
                um_sb = constp.tile([128, N], F16)
            for s in range(S):
                sl = slice(s * N, s * N + CW)
                nc.sync.dma_start(embTs_sb[:, sl], embTs_d[:, sl])
            nc.sync.dma_start(Wblk_sb[:, 0:GRP * S * 128], Wblk_d[:, 0:GRP * S * 128])
            if unmask:
                nc.sync.dma_start(um_sb[:, 0:CW], um_d[:, 0:CW])

            gb_sb = constp.tile([128, H], F16)
            nc.gpsimd.dma_start(gb_sb[:], gb_d)
            bb_sb = constp.tile([128, H], F16)
            nc.gpsimd.dma_start(bb_sb[:], bb_d)
            for g in range(1, NG):
                for s in range(S):
                    sl = slice(s * N + g * CW, s * N + (g + 1) * CW)
                    nc.gpsimd.dma_start(embTs_sb[:, sl], embTs_d[:, sl])
                wl = slice(g * GRP * S * 128, (g + 1) * GRP * S * 128)
                nc.gpsimd.dma_start(Wblk_sb[:, wl], Wblk_d[:, wl])
                if unmask:
                    nc.gpsimd.dma_start(
                        um_sb[:, g * CW:(g + 1) * CW], um_d[:, g * CW:(g + 1) * CW])
            muB_sb = constp.tile([1, M], F16)
            nc.gpsimd.dma_start(muB_sb[:], muB_d)
            id_sb = constp.tile([128, 128], F16)
            nc.gpsimd.dma_start(id_sb[:], id_d)
            nuB_sb = constp.tile([1, N], F16)
            nc.gpsimd.dma_start(nuB_sb[:], nuB_d)
            eps_sb = constp.tile([128, 1], F32)
            nc.vector.memset(eps_sb[:], LN_EPS)
            ones_sb = constp.tile([128, 1], F16)
            nc.vector.memset(ones_sb[:], 1.0)

            x_pre = xpool.tile([128, N], F16)     # merged projection + bias
            x_sb = xpool.tile([128, N], F16)      # LayerNorm output
            s_all = xpool.tile([128, NBLK], F32)
            q_all = xpool.tile([128, NBLK], F32)
            mu_all = xpool.tile([128, NBLK], F32)
            e2_all = xpool.tile([128, NBLK], F32)
            nv_all = xpool.tile([128, NBLK], F32)
            sd_all = xpool.tile([128, NBLK], F32)
            r_all = xpool.tile([128, NBLK], F32)
            colsum_sb = xpool.tile([1, 128], F16)
            lcs_sb = xpool.tile([1, 128], F16)

            # adjP loads: 2 blocks per DMA, all on the sync (SP) queue
            adj_tiles = []
            for kk in range(NBLK // 2):
                at = adjp.tile([128, 2 * M], F8E3, tag="adj")
                nc.sync.dma_start(at[:], adjP_d[:, kk * 2 * M:(kk + 1) * 2 * M])
                adj_tiles.append(at)
            # adjT loads issued NOW (prefetch during P2), split even/odd
            # between the sync and gpsimd queues so a blocked ring slot on
            # one never stalls the other stream.
            att_tiles = []
            for mb in range(MBLK):
                att = adjtp.tile([128, N], F8E3, tag="at")
                eng = nc.sync if mb % 2 == 0 else nc.gpsimd
                eng.dma_start(att[:], adjT_d[:, mb * N:(mb + 1) * N])
                att_tiles.append(att)

            # ------------- P1 + P2 (pipelined per block) ---------------------
            ps_lat_cm = tc.tile_pool(name="pslat", bufs=1, space="PSUM")
            ps_pp_cm = tc.tile_pool(name="pspp", bufs=2, space="PSUM")
            ps_cs_cm = tc.tile_pool(name="pscs", bufs=1, space="PSUM")
            ps_lat = ps_lat_cm.__enter__()
            ps_pp = ps_pp_cm.__enter__()
            ps_cs = ps_cs_cm.__enter__()

            cs_ps = ps_cs.tile([1, 128], F32)
            lat_ps = ps_lat.tile([128, M], F32)

            for g in range(NG):
                # projection matmuls: bias + S gated slots accumulate in PSUM.
                # 4 blocks share one PSUM bank; one wide evacuation per 4.
                for half in range(GRP // 4):
                    pp = ps_pp.tile([128, 512], F32, tag="pp")
                    for j4 in range(4):
                        k = g * GRP + half * 4 + j4
                        ksl = slice(k * 128, (k + 1) * 128)
                        psl = slice(j4 * 128, (j4 + 1) * 128)
                        nc.tensor.matmul(pp[:, psl], oneh_sb[:, ksl], btab_sb[:],
                                         start=True, stop=False)
                        for s in range(S):
                            last = (s == S - 1) and not unmask
                            nc.tensor.matmul(
                                pp[:, psl],
                                embTs_sb[:, s * N + k * 128:s * N + (k + 1) * 128],
                                Wblk_sb[:, (k * S + s) * 128:(k * S + s + 1) * 128],
                                start=False, stop=last)
                        if unmask:
                            nc.tensor.matmul(pp[:, psl], um_sb[:, ksl], id_sb[:],
                                             start=False, stop=True)
                    k0 = (g * GRP + half * 4) * 128
                    nc.scalar.copy(x_pre[:, k0:k0 + 512], pp[:])

                # batched LayerNorm stats for the group
                gsl = slice(g * CW, (g + 1) * CW)
                gbl = slice(g * GRP, (g + 1) * GRP)
                sq = sqp.tile([128, CW], F16, tag="sq")
                nc.vector.tensor_tensor(sq[:], x_pre[:, gsl], x_pre[:, gsl], op=MUL)
                nc.vector.tensor_reduce(
                    s_all[:, gbl],
                    x_pre[:, gsl].rearrange("p (k d) -> p k d", d=128),
                    axis=mybir.AxisListType.X, op=ADD)
                nc.vector.tensor_reduce(
                    q_all[:, gbl],
                    sq[:].rearrange("p (k d) -> p k d", d=128),
                    axis=mybir.AxisListType.X, op=ADD)
                nc.vector.tensor_scalar_mul(mu_all[:, gbl], s_all[:, gbl], 1.0 / H)
                nc.vector.tensor_scalar_mul(e2_all[:, gbl], q_all[:, gbl], 1.0 / H)
                nc.vector.tensor_tensor(nv_all[:, gbl], mu_all[:, gbl],
                                        mu_all[:, gbl], op=MUL)
                nc.vector.tensor_tensor(nv_all[:, gbl], nv_all[:, gbl],
                                        e2_all[:, gbl], op=SUB)
                nc.scalar.activation(sd_all[:, gbl], nv_all[:, gbl], AF.Sqrt,
                                     bias=eps_sb[:], scale=-1.0)
                nc.vector.reciprocal(r_all[:, gbl], sd_all[:, gbl])

                for j in range(GRP):
                    k = g * GRP + j
                    ksl = slice(k * 128, (k + 1) * 128)
                    a_t = atp.tile([128, 128], F16, tag="a")
                    nc.vector.scalar_tensor_tensor(
                        a_t[:], x_pre[:, ksl], mu_all[:, k:k + 1], gb_sb[:],
                        op0=SUB, op1=MUL)
                    nc.vector.scalar_tensor_tensor(
                        x_sb[:, ksl], a_t[:], r_all[:, k:k + 1], bb_sb[:],
                        op0=MUL, op1=ADD)
                    # colsum_x accumulation (for the exact mean-restore term)
                    nc.tensor.matmul(cs_ps[:], ones_sb[:], x_sb[:, ksl],
                                     start=(k == 0), stop=(k == NBLK - 1))
                    # P2 accumulation for this block
                    at = adj_tiles[k // 2]
                    base = (k % 2) * M
                    for c in range(MCH):
                        nc.tensor.matmul(
                            lat_ps[:, c * 512:(c + 1) * 512],
                            x_sb[:, ksl],
                            at[:, base + c * 512:base + (c + 1) * 512],
                            start=(k == 0), stop=False)

            nc.vector.tensor_copy(colsum_sb[:], cs_ps[:])
            for c in range(MCH):
                nc.tensor.matmul(
                    lat_ps[:, c * 512:(c + 1) * 512],
                    colsum_sb[:], muB_sb[:, c * 512:(c + 1) * 512],
                    start=False, stop=True)

            # ------------- lat evac + transpose + latcolsum ------------------
            latT_sb = xpool.tile([128, M], F16)
            for c in range(MCH):
                csl = slice(c * 512, (c + 1) * 512)
                nc.scalar.copy(latT_sb[:, csl], lat_ps[:, csl])
            lcs32 = xpool.tile([128, 1], F32)
            nc.vector.tensor_reduce(lcs32[:], latT_sb[:],
                                    axis=mybir.AxisListType.X, op=ADD)
            # scale by 1/16 so the fp16 stationary can't overflow (host
            # pre-scales nuB by 16 to compensate)
            lcsc = xpool.tile([128, 1], F16)
            nc.vector.tensor_scalar_mul(lcsc[:], lcs32[:], 1.0 / 16.0)

            ps_cs_cm.__exit__(None, None, None)
            ps_pp_cm.__exit__(None, None, None)

            lat_sb = xpool.tile([128, M], F16)
            pst_cm = tc.tile_pool(name="pst", bufs=2, space="PSUM")
            pst = pst_cm.__enter__()
            lt = pst.tile([1, 128], F16, tag="lt")
            nc.tensor.transpose(lt[:], lcsc[:], id_sb[:])
            nc.vector.tensor_copy(lcs_sb[:], lt[:])
            for q in range(MCH):
                pt = pst.tile([128, 512], F16, tag="pt")
                for jj in range(4):
                    mb = q * 4 + jj
                    nc.tensor.transpose(
                        pt[:, jj * 128:(jj + 1) * 128],
                        latT_sb[:, mb * 128:(mb + 1) * 128], id_sb[:])
                nc.vector.tensor_copy(lat_sb[:, q * 512:(q + 1) * 512], pt[:])
            pst_cm.__exit__(None, None, None)
            ps_lat_cm.__exit__(None, None, None)

            # ------------- P3: retT[h, n] ------------------------------------
            ps3_cm = tc.tile_pool(name="ps3", bufs=1, space="PSUM")
            ps3 = ps3_cm.__enter__()
            ret_ps = ps3.tile([128, N], F32)
            for c in range(NCH):
                nc.tensor.matmul(
                    ret_ps[:, c * 512:(c + 1) * 512],
                    lcs_sb[:], nuB_sb[:, c * 512:(c + 1) * 512],
                    start=True, stop=False)
            for mb in range(MBLK):
                att = adjtp.tile([128, N], F8E3, tag="at")
                nc.sync.dma_start(att[:], adjT_d[:, mb * N:(mb + 1) * N])
                for c in range(NCH):
                    nc.tensor.matmul(
                        ret_ps[:, c * 512:(c + 1) * 512],
                        lat_sb[:, mb * 128:(mb + 1) * 128],
                        att[:, c * 512:(c + 1) * 512],
                        start=False, stop=(mb == MBLK - 1))
            # evacuate scaled by 1/16: |ret| can reach ~2e5 > fp16 max.
            # host multiplies the output back by 16.
            rt = xpool.tile([128, N], F16)
            for c in range(NCH):
                csl = slice(c * 512, (c + 1) * 512)
                if c % 2 == 0:
                    nc.vector.tensor_scalar_mul(rt[:, csl], ret_ps[:, csl], 1.0 / 16.0)
                else:
                    nc.scalar.activation(rt[:, csl], ret_ps[:, csl], AF.Copy,
                                         scale=1.0 / 16.0)
            for hh in range(2):
                hsl = slice(hh * 2048, (hh + 1) * 2048)
                nc.sync.dma_start(ret_d[:, hsl], rt[:, hsl])
            ps3_cm.__exit__(None, None, None)

    nc.compile()
    nc.m = get_hw_module(nc.m)
    return nc


def _prep_core(adj_b, emb_b, type_b, mask_b, W16, b16, S, unmask):
    """Host-side input marshalling for one batch (sorted-node order)."""
    perm = np.argsort(type_b, kind="stable")
    t_s = type_b[perm]
    m_s = mask_b[perm]
    adj_p = adj_b[perm]                              # [N, M] f32

    mu16 = adj_p.mean(axis=0).astype(np.float16)
    d = (adj_p - mu16[None, :].astype(np.float32)).astype(E3)
    adjP = np.ascontiguousarray(
        d.reshape(NBLK, 128, M).transpose(1, 0, 2).reshape(128, NBLK * M))

    nu16 = adj_p.mean(axis=1).astype(np.float16)
    e = (adj_p - nu16[:, None].astype(np.float32)).T.astype(E3)   # [M, N]
    adjT = np.ascontiguousarray(
        e.reshape(MBLK, 128, N).transpose(1, 0, 2).reshape(128, MBLK * N))

    embT = np.ascontiguousarray(emb_b[perm].T.astype(np.float16))  # [H, N]

    embTs = np.zeros((128, S * N), np.float16)
    Wblk = np.zeros((128, NBLK * S * 128), np.float16)
    for k in range(NBLK):
        ksl = slice(k * 128, (k + 1) * 128)
        blk_t = t_s[ksl]
        blk_m = m_s[ksl]
        uniq = np.unique(blk_t[blk_m]) if not blk_m.all() else np.unique(blk_t)
        assert len(uniq) <= S
        for s, tt in enumerate(uniq):
            gate = (blk_t == tt) & blk_m
            embTs[:, s * N + k * 128:s * N + (k + 1) * 128] = \
                embT[:, ksl] * gate[None, :].astype(np.float16)
            Wblk[:, (k * S + s) * 128:(k * S + s + 1) * 128] = W16[tt]

    oneh = ((t_s[None, :] == np.arange(T)[:, None]) & m_s[None, :]).astype(np.float16)

    out = {
        "embTs": embTs, "Wblk": Wblk, "oneh": oneh,
        "adjP": adjP, "adjT": adjT,
        "muB": mu16.reshape(1, M),
        "nuB": (nu16.astype(np.float32) * 16.0).astype(np.float16).reshape(1, N),
    }
    if unmask:
        out["umT"] = embT * (~m_s)[None, :].astype(np.float16)
    return perm, out


def kernel(adj, embeds, node_type_index, node_mask, W, b, gamma, beta):
    adj = np.asarray(adj, np.float32)
    embeds = np.asarray(embeds, np.float32)
    node_type_index = np.asarray(node_type_index)
    node_mask = np.asarray(node_mask).astype(bool)
    W16 = np.asarray(W, np.float32).astype(np.float16)
    b16 = np.asarray(b, np.float32).astype(np.float16)
    gamma = np.asarray(gamma, np.float32)
    beta = np.asarray(beta, np.float32)

    unmask = not node_mask.all()

    # S = max distinct (masked) node types within any sorted 128-node block
    S = 1
    for bi in range(B):
        t_sorted = np.sort(node_type_index[bi])
        for k in range(NBLK):
            S = max(S, len(np.unique(t_sorted[k * 128:(k + 1) * 128])))

    common = {
        "btab": b16.reshape(T, H),
        "gb": np.ascontiguousarray(np.broadcast_to(gamma, (128, H))).astype(np.float16),
        "bb": np.ascontiguousarray(np.broadcast_to(beta, (128, H))).astype(np.float16),
        "ident": np.eye(128, dtype=np.float16),
    }
    perms = []
    in_maps = []
    for bi in range(B):
        perm, m = _prep_core(
            adj[bi], embeds[bi], node_type_index[bi], node_mask[bi],
            W16, b16, S, unmask)
        perms.append(perm)
        m.update(common)
        in_maps.append(m)

    key = (S, unmask)
    if key not in _PROGRAM_CACHE:
        _PROGRAM_CACHE[key] = _build_program(S, unmask)
    nc = _PROGRAM_CACHE[key]

    res = run_bass_kernel_spmd(nc, in_maps, core_ids=list(range(B)))
    kernel.last_results = res
    kernel.last_nc = nc

    out = np.empty((B, N, H), np.float32)
    for bi in range(B):
        ret_sorted = res.results[bi]["retT"].T.astype(np.float32) * 16.0   # [N, H]
        out[bi][perms[bi]] = ret_sorted
    return out



# revision 29
# speedup vs baseline: 1.8225x; 1.8225x over previous
"""Trainium2 Bass kernel for nn_HHGNN_17179869473.

Computation (per batch b):
    y     = embeds @ W[t_n] + b[t_n]         (per-type linear, routed by type)
    x     = where(mask, y, embeds); x = LayerNorm(x) * gamma + beta
    lat   = adj^T @ x                        [M, H]
    ret   = adj @ lat                        [N, H]

Strategy: pure data parallel, batch b -> core b (B == 8 == n_cores).

Key performance structure (v2):
  * Everything that feeds the PE is float8e4 (e4m3), so every heavy matmul
    runs in DoubleRow perf mode: one instruction contracts 256 rows (two
    128-deep k-tiles packed side by side in both operands).
  * adj is streamed as e4m3 in both layouts, column/row-mean centered on
    the host; the exact rank-1 mean terms are restored on device by cheap
    fp16 outer-product matmuls folded into each PSUM accumulation chain.
  * Host lays adjP/adjT out pair-chunk interleaved so each DoubleRow
    moving operand is a contiguous [128, 2*512] slice viewed as
    [128, 2, 512].
  * All input DMAs are issued on the sync (SP) HWDGE queue in priority
    order into fully resident SBUF tiles: consts, embTs/Wblk per group,
    adjP pairs, adjT half-pairs.  The DMA-engine device streams
    back-to-back; compute rides underneath.
  * LayerNorm: stats batched per 8-block group on DVE; the normalization
    itself is a single Activation-engine Copy per block with per-partition
    scale=1/sd and bias=-mu/sd (gamma==1, beta==0 fast path).
  * fp32 accumulation in PSUM everywhere; output evacuated scaled by 1/16
    to fp16 (host multiplies back).
"""

import os
import sys

for _p in ("/opt/trn_rl_repo", "/root/.axon_site/_ro/trn_rl_repo"):
    if os.path.isdir(_p) and _p not in sys.path:
        sys.path.insert(0, _p)

import numpy as np
import ml_dtypes

import concourse.bacc as bacc
import concourse.tile as tile
from concourse import mybir
from concourse.bass_interp import get_hw_module
from concourse.bass_utils import run_bass_kernel_spmd

B, N, M, H, T = 8, 4096, 2048, 128, 13
NBLK = N // 128          # 32 node blocks
MBLK = M // 128          # 16 m blocks
NPAIR = NBLK // 2        # 16 node-block pairs (DoubleRow k-tiles)
MPAIR = MBLK // 2        # 8 m-block pairs
MCH = M // 512           # 4 chunks over m
NCH = N // 512           # 8 chunks over n
GRP = 8                  # blocks per LayerNorm-stats group
NG = NBLK // GRP
LN_EPS = 1e-5
F32 = mybir.dt.float32
F16 = mybir.dt.float16
F8E4 = mybir.dt.float8e4
DR = mybir.MatmulPerfMode.DoubleRow
ADD = mybir.AluOpType.add
SUB = mybir.AluOpType.subtract
MUL = mybir.AluOpType.mult
AF = mybir.ActivationFunctionType
E4 = ml_dtypes.float8_e4m3

_PROGRAM_CACHE = {}


def _build_program(S, unmask, ln_trivial):
    """SPMD-uniform Bass program.  S = type slots per block (even, >= 2);
    unmask = True adds the (1-mask)*embeds passthrough term; ln_trivial =
    gamma==1 and beta==0 (single-activation LayerNorm apply)."""
    assert S % 2 == 0 and S >= 2
    nc = bacc.Bacc(
        "TRN2",
        target_bir_lowering=False,
        debug=False,
        enable_asserts=False,
        num_devices=B,
    )

    SW = NBLK * S * 128
    embTs_d = nc.dram_tensor("embTs", [128, SW], F8E4, kind="ExternalInput").ap()
    Wblk_d = nc.dram_tensor("Wblk", [128, SW], F8E4, kind="ExternalInput").ap()
    oneh_d = nc.dram_tensor("oneh", [T, N], F8E4, kind="ExternalInput").ap()
    btab_d = nc.dram_tensor("btab", [T, H], F8E4, kind="ExternalInput").ap()
    adjP_d = nc.dram_tensor("adjP", [128, NBLK * M], F8E4, kind="ExternalInput").ap()
    adjT_d = nc.dram_tensor("adjT", [128, MBLK * N], F8E4, kind="ExternalInput").ap()
    muB_d = nc.dram_tensor("muB", [1, M], F16, kind="ExternalInput").ap()
    nuB_d = nc.dram_tensor("nuB", [1, N], F16, kind="ExternalInput").ap()
    # host-exact colsum of x (fp32 LN of the fp32 projection): using it for
    # the P2 mean-restore cancels the correlated fp8 quantization error of
    # the x / embTs / Wblk tensors, which otherwise lands on the output's
    # dominant rank-1 component at full relative magnitude (~3%)
    csB_d = nc.dram_tensor("csB", [1, H], F16, kind="ExternalInput").ap()
    id16_d = nc.dram_tensor("id16", [128, 128], F16, kind="ExternalInput").ap()
    if not ln_trivial:
        gb_d = nc.dram_tensor("gb", [128, H], F16, kind="ExternalInput").ap()
        bb_d = nc.dram_tensor("bb", [128, H], F16, kind="ExternalInput").ap()
    if unmask:
        um_d = nc.dram_tensor("umT", [128, N], F16, kind="ExternalInput").ap()
    ret_d = nc.dram_tensor("retT", [128, N], F16, kind="ExternalOutput").ap()

    def pair2(ap_slice):
        return ap_slice.rearrange("p (two f) -> p two f", two=2)

    with tile.TileContext(nc, trace_sim=False) as tc:
        with (
            tc.tile_pool(name="const", bufs=1) as constp,
            tc.tile_pool(name="xpool", bufs=1) as xpool,
            tc.tile_pool(name="sqp", bufs=2) as sqp,
            tc.tile_pool(name="atp", bufs=4) as atp,
        ):
            # ---------- input DMAs ----------
            # sync (SP/HWDGE) queue carries the critical stream in priority
            # order: P1 consts, embTs/Wblk, adjP pairs, adjT half-pairs
            # (chunks 0-3 of every pair first, then chunks 4-7, so the
            # first output half can store while the second half streams).
            # Small consts not needed until P2's tail ride the gpsimd
            # (SWDGE) queue.
            embTs_sb = constp.tile([128, SW], F8E4)
            nc.sync.dma_start(embTs_sb[:], embTs_d)
            Wblk_sb = constp.tile([128, SW], F8E4)
            nc.sync.dma_start(Wblk_sb[:], Wblk_d)
            oneh_sb = constp.tile([T, N], F8E4)
            nc.sync.dma_start(oneh_sb[:], oneh_d)
            btab_sb = constp.tile([T, H], F8E4)
            nc.sync.dma_start(btab_sb[:], btab_d)
            if unmask:
                um_sb = constp.tile([128, N], F16)
                nc.sync.dma_start(um_sb[:], um_d)

            adjP_sb = constp.tile([128, NBLK * M], F8E4)
            for j in range(NPAIR):
                sl = slice(j * 2 * M, (j + 1) * 2 * M)
                nc.sync.dma_start(adjP_sb[:, sl], adjP_d[:, sl])
            # adjT: chunks 0-3 of every pair first (half-0 of P3), then
            # chunks 4-7; the very last pair arrives chunk-by-chunk so the
            # final stop-matmuls start as early as possible.
            adjT_sb = constp.tile([128, MBLK * N], F8E4)
            for hf in range(2):
                for q in range(MPAIR):
                    base = q * 2 * N + hf * 4 * 1024
                    if hf == 1 and q == MPAIR - 1:
                        for cc in range(4):
                            sl = slice(base + cc * 1024, base + (cc + 1) * 1024)
                            nc.sync.dma_start(adjT_sb[:, sl], adjT_d[:, sl])
                    else:
                        sl = slice(base, base + 4 * 1024)
                        nc.sync.dma_start(adjT_sb[:, sl], adjT_d[:, sl])

            id16_sb = constp.tile([128, 128], F16)
            nc.gpsimd.dma_start(id16_sb[:], id16_d)
            csB_sb = constp.tile([1, H], F16)
            nc.gpsimd.dma_start(csB_sb[:], csB_d)
            muB_sb = constp.tile([1, M], F16)
            nc.gpsimd.dma_start(muB_sb[:], muB_d)
            nuB_sb = constp.tile([1, N], F16)
            nc.gpsimd.dma_start(nuB_sb[:], nuB_d)
            if not ln_trivial:
                gb_sb = constp.tile([128, H], F16)
                nc.gpsimd.dma_start(gb_sb[:], gb_d)
                bb_sb = constp.tile([128, H], F16)
                nc.gpsimd.dma_start(bb_sb[:], bb_d)

            eps_sb = constp.tile([128, 1], F32)
            nc.vector.memset(eps_sb[:], LN_EPS)

            x_pre = xpool.tile([128, N], F16)     # merged projection + bias
            x_sb = xpool.tile([128, N], F8E4)     # LayerNorm output
            s_all = xpool.tile([128, NBLK], F32)
            q_all = xpool.tile([128, NBLK], F32)
            mu_all = xpool.tile([128, NBLK], F32)
            e2_all = xpool.tile([128, NBLK], F32)
            nv_all = xpool.tile([128, NBLK], F32)
            sd_all = xpool.tile([128, NBLK], F32)
            r_all = xpool.tile([128, NBLK], F32)
            mr_all = xpool.tile([128, NBLK], F32)
            lcs_sb = xpool.tile([1, 128], F16)

            # ------------- P1 + P2 (pipelined per group) ---------------------
            ps_lat_cm = tc.tile_pool(name="pslat", bufs=1, space="PSUM")
            ps_pp_cm = tc.tile_pool(name="pspp", bufs=2, space="PSUM")
            ps_lat = ps_lat_cm.__enter__()
            ps_pp = ps_pp_cm.__enter__()

            lat_ps = ps_lat.tile([128, M], F32)

            CW = GRP * 128
            for g in range(NG):
                # projection matmuls: bias + S/2 DoubleRow slot-pair matmuls
                # accumulate in PSUM.  4 blocks share one PSUM bank.
                for half in range(GRP // 4):
                    pp = ps_pp.tile([128, 512], F32, tag="pp")
                    for j4 in range(4):
                        k = g * GRP + half * 4 + j4
                        ksl = slice(k * 128, (k + 1) * 128)
                        psl = slice(j4 * 128, (j4 + 1) * 128)
                        # DoubleRow slot pairs start the chain (embTs/Wblk
                        # arrive before oneh/btab in the DMA stream); the
                        # routed-bias matmul closes it.
                        for u in range(S // 2):
                            wsl = slice((k * S + 2 * u) * 128,
                                        (k * S + 2 * u + 2) * 128)
                            nc.tensor.matmul(
                                pp[:, psl],
                                pair2(embTs_sb[:, wsl]),
                                pair2(Wblk_sb[:, wsl]),
                                start=(u == 0), stop=False, perf_mode=DR)
                        if unmask:
                            nc.tensor.matmul(pp[:, psl], um_sb[:, ksl], id16_sb[:],
                                             start=False, stop=False)
                        nc.tensor.matmul(pp[:, psl], oneh_sb[:, ksl], btab_sb[:],
                                         start=False, stop=True)
                    k0 = (g * GRP + half * 4) * 128
                    nc.scalar.copy(x_pre[:, k0:k0 + 512], pp[:])

                # batched LayerNorm stats for the group
                gsl = slice(g * CW, (g + 1) * CW)
                gbl = slice(g * GRP, (g + 1) * GRP)
                sq = sqp.tile([128, CW], F16, tag="sq")
                nc.vector.tensor_tensor(sq[:], x_pre[:, gsl], x_pre[:, gsl], op=MUL)
                nc.vector.tensor_reduce(
                    s_all[:, gbl],
                    x_pre[:, gsl].rearrange("p (k d) -> p k d", d=128),
                    axis=mybir.AxisListType.X, op=ADD)
                nc.vector.tensor_reduce(
                    q_all[:, gbl],
                    sq[:].rearrange("p (k d) -> p k d", d=128),
                    axis=mybir.AxisListType.X, op=ADD)
                nc.vector.tensor_scalar_mul(mu_all[:, gbl], s_all[:, gbl], 1.0 / H)
                nc.vector.tensor_scalar_mul(e2_all[:, gbl], q_all[:, gbl], 1.0 / H)
                nc.vector.tensor_tensor(nv_all[:, gbl], mu_all[:, gbl],
                                        mu_all[:, gbl], op=MUL)
                nc.vector.tensor_tensor(nv_all[:, gbl], nv_all[:, gbl],
                                        e2_all[:, gbl], op=SUB)
                nc.scalar.activation(sd_all[:, gbl], nv_all[:, gbl], AF.Sqrt,
                                     bias=eps_sb[:], scale=-1.0)
                nc.vector.reciprocal(r_all[:, gbl], sd_all[:, gbl])
                nc.vector.tensor_tensor(mr_all[:, gbl], mu_all[:, gbl],
                                        r_all[:, gbl], op=MUL)
                nc.vector.tensor_scalar_mul(mr_all[:, gbl], mr_all[:, gbl], -1.0)

                for j in range(GRP):
                    k = g * GRP + j
                    ksl = slice(k * 128, (k + 1) * 128)
                    if ln_trivial:
                        # x = (x_pre - mu) / sd   in one ACT op
                        nc.scalar.activation(
                            x_sb[:, ksl], x_pre[:, ksl], AF.Identity,
                            bias=mr_all[:, k:k + 1], scale=r_all[:, k:k + 1])
                    else:
                        a_t = atp.tile([128, 128], F16, tag="a")
                        nc.scalar.activation(
                            a_t[:], x_pre[:, ksl], AF.Identity,
                            bias=mr_all[:, k:k + 1], scale=r_all[:, k:k + 1])
                        b_t = atp.tile([128, 128], F16, tag="b")
                        nc.vector.tensor_tensor(b_t[:], a_t[:], gb_sb[:], op=MUL)
                        nc.vector.tensor_tensor(x_sb[:, ksl], b_t[:], bb_sb[:],
                                                op=ADD)

                # P2 per block pair, DoubleRow
                for jp in range(GRP // 2):
                    j = g * (GRP // 2) + jp
                    xsl = slice(j * 256, (j + 1) * 256)
                    for c in range(MCH):
                        asl = slice((j * MCH + c) * 1024, (j * MCH + c + 1) * 1024)
                        nc.tensor.matmul(
                            lat_ps[:, c * 512:(c + 1) * 512],
                            pair2(x_sb[:, xsl]),
                            pair2(adjP_sb[:, asl]),
                            start=(j == 0), stop=False, perf_mode=DR)

            # exact mean restore: lat += host-exact colsum_x (outer) muB
            for c in range(MCH):
                nc.tensor.matmul(
                    lat_ps[:, c * 512:(c + 1) * 512],
                    csB_sb[:], muB_sb[:, c * 512:(c + 1) * 512],
                    start=False, stop=True)

            # ------------- lat evac + transpose + latcolsum ------------------
            # latT/transposes stay fp16 (walrus rejects fp8 PE-transpose
            # outputs with unit element step); lat_sb converts to e4 at the
            # PSUM evacuation
            latT_sb = xpool.tile([128, M], F16)
            for c in range(MCH):
                csl = slice(c * 512, (c + 1) * 512)
                if c % 2 == 0:
                    nc.scalar.copy(latT_sb[:, csl], lat_ps[:, csl])
                else:
                    nc.vector.tensor_copy(latT_sb[:, csl], lat_ps[:, csl])
            lcs32 = xpool.tile([128, 1], F32)
            nc.vector.tensor_reduce(lcs32[:], latT_sb[:],
                                    axis=mybir.AxisListType.X, op=ADD)
            # lat_sb carries lat/4 (|lat| can exceed e4m3's 240 max), so the
            # nu-restore term needs lcs/4; the extra 1/16 keeps the fp16
            # stationary in range (host pre-scales nuB by 16 to compensate)
            lcsc = xpool.tile([128, 1], F16)
            nc.vector.tensor_scalar_mul(lcsc[:], lcs32[:], 1.0 / 64.0)

            ps_pp_cm.__exit__(None, None, None)

            lat_sb = xpool.tile([128, M], F8E4)
            pst_cm = tc.tile_pool(name="pst", bufs=2, space="PSUM")
            pst = pst_cm.__enter__()
            lt = pst.tile([1, 128], F16, tag="lt")
            nc.tensor.transpose(lt[:], lcsc[:], id16_sb[:])
            nc.vector.tensor_copy(lcs_sb[:], lt[:])
            for q in range(MCH):
                pt = pst.tile([128, 512], F16, tag="pt")
                for jj in range(4):
                    mb = q * 4 + jj
                    nc.tensor.transpose(
                        pt[:, jj * 128:(jj + 1) * 128],
                        latT_sb[:, mb * 128:(mb + 1) * 128], id16_sb[:])
                if q % 2 == 0:
                    nc.vector.tensor_scalar_mul(
                        lat_sb[:, q * 512:(q + 1) * 512], pt[:], 0.25)
                else:
                    nc.scalar.activation(
                        lat_sb[:, q * 512:(q + 1) * 512], pt[:],
                        AF.Copy, scale=0.25)
            pst_cm.__exit__(None, None, None)
            ps_lat_cm.__exit__(None, None, None)

            # ------------- P3: retT[h, n], DoubleRow over m pairs ------------
            # two n-halves: half 0 consumes chunks 0-3 of every adjT pair
            # (the first 8 adjT DMAs) and stores its output while half 1's
            # adjT stream is still in flight.
            ps3_cm = tc.tile_pool(name="ps3", bufs=1, space="PSUM")
            ps3 = ps3_cm.__enter__()
            # one PSUM tile per 512-col chunk (no false tile-granular
            # coupling between the final matmuls and the evacuations); rt
            # tiles pair two chunks so each store is one 1024-col DMA
            rp_c = [ps3.tile([128, 512], F32, name=f"rp{i}")
                    for i in range(NCH)]
            rt_c = [xpool.tile([128, 1024], F16, name=f"rt{i}")
                    for i in range(NCH // 2)]
            HC = NCH // 2
            for hf in range(2):
                for c in range(HC):
                    ch = hf * HC + c
                    nc.tensor.matmul(
                        rp_c[ch][:],
                        lcs_sb[:], nuB_sb[:, ch * 512:(ch + 1) * 512],
                        start=True, stop=False)
                for q in range(MPAIR):
                    lsl = slice(q * 256, (q + 1) * 256)
                    last = (q == MPAIR - 1)
                    for c in range(HC):
                        ch = hf * HC + c
                        asl = slice((q * NCH + ch) * 1024,
                                    (q * NCH + ch + 1) * 1024)
                        nc.tensor.matmul(
                            rp_c[ch][:],
                            pair2(lat_sb[:, lsl]),
                            pair2(adjT_sb[:, asl]),
                            start=False, stop=last, perf_mode=DR)
                # evacuate scaled by 1/16 (|ret| can exceed fp16 max; host
                # multiplies the output back by 16), alternating engines;
                # store each 512-col chunk as soon as it is ready.
                for c in range(HC):
                    ch = hf * HC + c
                    rt = rt_c[ch // 2]
                    hsl = slice((ch % 2) * 512, (ch % 2) * 512 + 512)
                    if c % 2 == 0:
                        nc.vector.tensor_scalar_mul(rt[:, hsl], rp_c[ch][:],
                                                    1.0 / 16.0)
                    else:
                        nc.scalar.activation(rt[:, hsl], rp_c[ch][:],
                                             AF.Copy, scale=1.0 / 16.0)
                        # store the 1024-col piece once both evacuations
                        # (DVE low half, ACT high half) are done
                        ssl = slice(ch * 512 - 512, (ch + 1) * 512)
                        nc.sync.dma_start(ret_d[:, ssl], rt[:])
            ps3_cm.__exit__(None, None, None)

    nc.compile()
    nc.m = get_hw_module(nc.m)
    return nc


def _prep_core(adj_b, emb_b, type_b, mask_b, W8, W32, b32, gamma, beta, S,
               unmask):
    """Host-side input marshalling for one batch (sorted-node order)."""
    perm = np.argsort(type_b, kind="stable")
    t_s = type_b[perm]
    m_s = mask_b[perm]
    adj_p = adj_b[perm]                              # [N, M] f32

    # exact (fp32) x = LayerNorm(where(mask, emb @ W[t] + b[t], emb)) and its
    # node-sum, for the device's P2 mean-restore term
    y = np.einsum("nh,nhd->nd", emb_b, W32[type_b]) + b32[type_b]
    xe = np.where(mask_b[:, None], y, emb_b)
    xe = (xe - xe.mean(-1, keepdims=True)) / np.sqrt(
        xe.var(-1) + LN_EPS)[:, None]
    xe = xe * gamma[None, :] + beta[None, :]
    csB = xe.sum(axis=0).astype(np.float16).reshape(1, H)

    mu16 = adj_p.mean(axis=0).astype(np.float16)
    d = (adj_p - mu16[None, :].astype(np.float32)).astype(E4)
    # pair-chunk interleaved: [128, pair j][chunk c][two b][512]
    adjP = np.ascontiguousarray(
        d.reshape(NPAIR, 2, 128, MCH, 512)
        .transpose(2, 0, 3, 1, 4).reshape(128, NBLK * M))

    nu16 = adj_p.mean(axis=1).astype(np.float16)
    e = (adj_p - nu16[:, None].astype(np.float32)).T.astype(E4)   # [M, N]
    adjT = np.ascontiguousarray(
        e.reshape(MPAIR, 2, 128, NCH, 512)
        .transpose(2, 0, 3, 1, 4).reshape(128, MBLK * N))

    embT = np.ascontiguousarray(emb_b[perm].T.astype(np.float16))  # [H, N]

    embTs = np.zeros((128, NBLK * S * 128), E4)
    Wblk = np.zeros((128, NBLK * S * 128), E4)
    for k in range(NBLK):
        ksl = slice(k * 128, (k + 1) * 128)
        blk_t = t_s[ksl]
        blk_m = m_s[ksl]
        uniq = np.unique(blk_t[blk_m]) if not blk_m.all() else np.unique(blk_t)
        assert len(uniq) <= S
        for s, tt in enumerate(uniq):
            gate = (blk_t == tt) & blk_m
            ssl = slice((k * S + s) * 128, (k * S + s + 1) * 128)
            embTs[:, ssl] = (embT[:, ksl]
                             * gate[None, :].astype(np.float16)).astype(E4)
            Wblk[:, ssl] = W8[tt]

    oneh = ((t_s[None, :] == np.arange(T)[:, None]) & m_s[None, :]).astype(E4)

    out = {
        "embTs": embTs, "Wblk": Wblk, "oneh": oneh,
        "adjP": adjP, "adjT": adjT,
        "muB": mu16.reshape(1, M),
        "nuB": (nu16.astype(np.float32) * 16.0).astype(np.float16).reshape(1, N),
        "csB": csB,
    }
    if unmask:
        out["umT"] = embT * (~m_s)[None, :].astype(np.float16)
    return perm, out


def kernel(adj, embeds, node_type_index, node_mask, W, b, gamma, beta):
    adj = np.asarray(adj, np.float32)
    embeds = np.asarray(embeds, np.float32)
    node_type_index = np.asarray(node_type_index)
    node_mask = np.asarray(node_mask).astype(bool)
    W8 = np.asarray(W, np.float32).astype(E4)
    b8 = np.asarray(b, np.float32).astype(E4)
    gamma = np.asarray(gamma, np.float32)
    beta = np.asarray(beta, np.float32)

    unmask = not node_mask.all()
    ln_trivial = bool(np.all(gamma == 1.0) and np.all(beta == 0.0))

    # S = max distinct (masked) node types within any sorted 128-node block,
    # rounded up to even (DoubleRow processes slots in pairs)
    S = 1
    for bi in range(B):
        t_sorted = np.sort(node_type_index[bi])
        for k in range(NBLK):
            S = max(S, len(np.unique(t_sorted[k * 128:(k + 1) * 128])))
    S = max(2, S + (S % 2))

    common = {
        "btab": b8.reshape(T, H),
        "id16": np.eye(128, dtype=np.float16),
    }
    if not ln_trivial:
        common["gb"] = np.ascontiguousarray(
            np.broadcast_to(gamma, (128, H))).astype(np.float16)
        common["bb"] = np.ascontiguousarray(
            np.broadcast_to(beta, (128, H))).astype(np.float16)
    perms = []
    in_maps = []
    for bi in range(B):
        perm, m = _prep_core(
            adj[bi], embeds[bi], node_type_index[bi], node_mask[bi],
            W8, np.asarray(W, np.float32), np.asarray(b, np.float32),
            gamma, beta, S, unmask)
        perms.append(perm)
        m.update(common)
        in_maps.append(m)

    key = (S, unmask, ln_trivial)
    if key not in _PROGRAM_CACHE:
        _PROGRAM_CACHE[key] = _build_program(S, unmask, ln_trivial)
    nc = _PROGRAM_CACHE[key]

    res = run_bass_kernel_spmd(nc, in_maps, core_ids=list(range(B)))
    kernel.last_results = res
    kernel.last_nc = nc

    out = np.empty((B, N, H), np.float32)
    for bi in range(B):
        # device computes ret/64 (1/4 lat prescale * 1/16 output evac scale)
        ret_sorted = res.results[bi]["retT"].T.astype(np.float32) * 64.0   # [N, H]
        out[bi][perms[bi]] = ret_sorted
    return out
